# revision 8
# baseline (speedup 1.0000x reference)
"""ConvTranspose3d(64->32, k=3, stride=2, pad=1, out_pad=1, dilation=2) on 8 NeuronCores.

Math: with stride=2, dilation=2, padding=1, k=3, every populated output
position o = 2i + 2k - 1 is odd in all three spatial dims, so the transposed
conv collapses to a dense 3^3 conv y[m] = sum_k wc[k] * x[m+k-1] on the 32^3
grid (wc = flip(transpose(w))) scattered into the odd sub-lattice of the 66^3
output; every other output voxel is exactly bias.

Device kernel (per core = one (batch, 8-plane depth slab)): for each of its 8
input planes d it computes three 2D-conv partials
    P_kd[co, oh, ow] = sum_{ci,kh,kw} wc[co,ci,kd,kh,kw] * xp[d, oh+kh, ow+kw]
as an implicit GEMM with M = (co*3+kd) = 96 PSUM partitions and K = (64 ci x
2 row-shifted input copies) = 128 SBUF partitions; the row-shifted copy is the
same contiguous DRAM bytes DMA'd at element offset +35 (one padded row), so
taps (kh=0,kh=1) pair up in K and each (plane, h-half) needs only 6 matmuls
of N=512 (3 full A-matmuls kh={0,1} + 3 half B-matmuls kh=2) instead of the
13.5 the depth-Toeplitz formulation needed. No depth halo is loaded; the host
combines P_kd across planes/cores (y[m] = P_0[m-1] + P_1[m] + P_2[m+1]), adds
bias, and scatters into the odd sub-lattice of the 66^3 canvas.
"""

import sys

sys.path.insert(0, "/opt/trn_rl_repo")

import numpy as np

N_CORES = 8
D_BLOCKS = 4   # depth blocks per batch
G_PER_CORE = 8  # input planes per core
PLANE = 35 * 35  # padded plane stride in elements

_cache = {}


def _build_nc():
    import concourse.bass as bass
    import concourse.tile as tile
    from concourse import bacc, mybir

    dt = mybir.dt
    nc = bacc.Bacc("TRN2", target_bir_lowering=False, debug=False,
                   num_devices=N_CORES)

    # xsf[ci, d*1225 + r*35 + w] = padded plane pixel; trailing 35 zeros so
    # the +35 row-shifted alias stays in bounds.
    xsf = nc.dram_tensor("xsf", [64, 8 * PLANE + 35], dt.bfloat16,
                         kind="ExternalInput")
    # 6 stationaries: s = kw*2 + {0:A (kh=0,1 pair), 1:B (kh=2, half)}
    # rows (j*64+ci), cols (co*3+kd)
    twf = nc.dram_tensor("twf", [128, 6 * 96], dt.bfloat16,
                         kind="ExternalInput")
    # P partials: [co*3+kd, d, hh, 512]
    out = nc.dram_tensor("out", [96, 8, 2, 512], dt.bfloat16,
                         kind="ExternalOutput")

    with tile.TileContext(nc) as tc:
        with (
            tc.tile_pool(name="tw", bufs=1) as tw_pool,
            tc.tile_pool(name="xt", bufs=1) as xt_pool,
            tc.tile_pool(name="ob", bufs=4) as ob_pool,
            tc.tile_pool(name="ps", bufs=8, space="PSUM") as ps_pool,
        ):
            tw_t = tw_pool.tile([128, 6 * 96], dt.bfloat16)
            # partitions 0:64 = xp, 64:128 = xp shifted one padded row (+35)
            xt = xt_pool.tile([128, 8, PLANE], dt.bfloat16)

            nc.sync.dma_start(tw_t[:, 0:96], twf[:, 0:96])
            xf = xt[:].rearrange("p d e -> p (d e)")
            for d in range(8):
                lo, hi = d * PLANE, (d + 1) * PLANE
                nc.scalar.dma_start(xf[0:64, lo:hi], xsf[:, lo:hi])
                nc.sync.dma_start(xf[64:128, lo:hi],
                                  xsf[:, lo + 35:hi + 35])
            nc.sync.dma_start(tw_t[:, 96:6 * 96], twf[:, 96:6 * 96])

            xv = xt[:].rearrange("p d (r w) -> p d r w", r=35)

            prev_last_mm = None
            for d in range(8):
                for hh in range(2):
                    r0 = 16 * hh
                    ps = ps_pool.tile([96, 16, 32], dt.float32)
                    for i in range(6):
                        kw, ab = i // 2, i % 2
                        rr = r0 + (0 if ab == 0 else 2)
                        s = kw * 2 + ab
                        rhs = xv[:, d, rr:rr + 16, kw:kw + 32]
                        mm = nc.tensor.matmul(ps[:],
                                              tw_t[:, s * 96:(s + 1) * 96],
                                              rhs, start=(i == 0),
                                              stop=(i == 5))
                        # keep the PE's static order group-contiguous so the
                        # flush fires right after each group's 6th matmul
                        if i == 0 and prev_last_mm is not None:
                            tile.add_dep_helper(
                                mm.ins, prev_last_mm.ins, sync=False,
                                reason="group-contiguous PE order")
                    prev_last_mm = mm
                    ob = ob_pool.tile([96, 512], dt.bfloat16)
                    nc.vector.tensor_scalar_add(
                        ob[:], ps[:].rearrange("p r w -> p (r w)"), 0.0)
                    nc.scalar.dma_start(out[:, d, hh, :], ob[:])

    nc.compile()
    return nc


def _prep_twf(weight):
    import ml_dtypes

    # wc[co, ci, kd, kh, kw] = weight[ci, co, 2-kd, 2-kh, 2-kw]
    wc = np.flip(np.transpose(weight, (1, 0, 2, 3, 4)), axis=(2, 3, 4))
    twf = np.zeros((128, 6, 32, 3), np.float32)
    for kw in range(3):
        for j in range(2):  # A: rows j*64+ci <- kh=j
            twf[j * 64:(j + 1) * 64, kw * 2, :, :] = \
                wc[:, :, :, j, kw].transpose(1, 0, 2)
        # B: rows 0:64 <- kh=2, rows 64:128 stay zero
        twf[0:64, kw * 2 + 1, :, :] = wc[:, :, :, 2, kw].transpose(1, 0, 2)
    return np.ascontiguousarray(
        twf.reshape(128, 6 * 96)).astype(ml_dtypes.bfloat16)


def _make_slab(x, n, cblk):
    import ml_dtypes

    xs = np.zeros((64, 8 * PLANE + 35), np.float32)
    v = xs[:, :8 * PLANE].reshape(64, 8, 35, 35)
    v[:, :, 1:33, 1:33] = x[n, :, 8 * cblk:8 * cblk + 8]
    return xs.astype(ml_dtypes.bfloat16)


def kernel(x, weight, bias):
    from concourse.bass_utils import run_bass_kernel_spmd

    if "nc" not in _cache:
        _cache["nc"] = _build_nc()
    nc = _cache["nc"]

    x = np.asarray(x, np.float32)
    weight = np.asarray(weight, np.float32)
    bias = np.asarray(bias, np.float32)

    twf = _prep_twf(weight)
    in_maps = []
    for core in range(N_CORES):
        n, cblk = divmod(core, D_BLOCKS)
        in_maps.append({"xsf": _make_slab(x, n, cblk), "twf": twf})

    res = run_bass_kernel_spmd(nc, in_maps, core_ids=list(range(N_CORES)))

    # per-batch global partials P[n, co, kd, d, h, w]
    P = np.empty((2, 32, 3, 32, 32, 32), np.float32)
    for core in range(N_CORES):
        n, cblk = divmod(core, D_BLOCKS)
        arr = np.asarray(res.results[core]["out"], dtype=np.float32)
        # [96, 8, 2, 512] -> (co, kd, d, hh, r, w) -> (co, kd, d, 32, 32)
        arr = arr.reshape(32, 3, 8, 2, 16, 32).reshape(32, 3, 8, 32, 32)
        P[n, :, :, 8 * cblk:8 * cblk + 8] = arr

    # y[m] = P_0[m-1] + P_1[m] + P_2[m+1]
    y = P[:, :, 1].copy()
    y[:, :, 1:] += P[:, :, 0, :31]
    y[:, :, :31] += P[:, :, 2, 1:]
    y += bias[None, :, None, None, None]

    # host assembly: everything except the odd sub-lattice is exactly bias
    full = np.empty((2, 32, 66, 66, 66), np.float32)
    full[...] = bias[None, :, None, None, None]
    full[:, :, 1:64:2, 1:64:2, 1:64:2] = y
    return full


# revision 45
# speedup vs baseline: 1.2545x; 1.2545x over previous
"""ConvTranspose3d(64->32, k=3, stride=2, pad=1, out_pad=1, dilation=2) on 8 NeuronCores.

Math: with stride=2, dilation=2, padding=1, k=3, every populated output
position o = 2i + 2k - 1 is odd in all three spatial dims, so the transposed
conv collapses to a dense 3^3 conv y[m] = sum_k wc[k] * x[m+k-1] on the 32^3
grid (wc = flip(transpose(w))) scattered into the odd sub-lattice of the 66^3
output; every other output voxel is exactly bias.

Device kernel (per core = one (batch, 8-plane depth slab)): for each of its 8
input planes d it computes three 2D-conv partials
    P_kd[co, oh, ow] = sum_{ci,kh,kw} wc[co,ci,kd,kh,kw] * xp[d, oh+kh, ow+kw]
as an implicit GEMM with M = (co*3+kd) = 96 PSUM partitions and K = (64 ci x
2 row-shifted input copies) = 128 SBUF partitions; the row-shifted copy is the
same contiguous DRAM bytes DMA'd at element offset +35 (one padded row), so
taps (kh=0,kh=1) pair up in K and each (plane, h-half) needs only 6 matmuls
of N=512 (3 full A-matmuls kh={0,1} + 3 half B-matmuls kh=2) instead of the
13.5 the depth-Toeplitz formulation needed. No depth halo is loaded; the host
combines P_kd across planes/cores (y[m] = P_0[m-1] + P_1[m] + P_2[m+1]), adds
bias, and scatters into the odd sub-lattice of the 66^3 canvas.
"""

import sys

sys.path.insert(0, "/opt/trn_rl_repo")

import numpy as np

N_CORES = 8
D_BLOCKS = 4   # depth blocks per batch
G_PER_CORE = 8  # input planes per core
PLANE = 35 * 35  # padded plane stride in elements

_cache = {}


def _build_nc():
    import concourse.bass as bass
    import concourse.tile as tile
    from concourse import bacc, mybir

    dt = mybir.dt
    nc = bacc.Bacc("TRN2", target_bir_lowering=False, debug=False,
                   num_devices=N_CORES)

    # xsf[ci, d*1225 + r*35 + w] = padded plane pixel; trailing 35 zeros so
    # the +35 row-shifted alias stays in bounds.
    xsf = nc.dram_tensor("xsf", [64, 8 * PLANE + 35], dt.bfloat16,
                         kind="ExternalInput")
    # 6 stationaries, cols (co*3+kd): s=0..2 A(kw) rows (j*64+ci) = tap
    # (kh=j, kw); s=3..5 B(kw) rows 0:64 = tap (kh=2, kw), K=64 only
    twf = nc.dram_tensor("twf", [128, 6 * 96], dt.bfloat16,
                         kind="ExternalInput")
    # P partials: [co*3+kd, d, hh, 512]
    out = nc.dram_tensor("out", [96, 8, 2, 512], dt.bfloat16,
                         kind="ExternalOutput")

    with tile.TileContext(nc) as tc:
        with (
            tc.tile_pool(name="tw", bufs=1) as tw_pool,
            tc.tile_pool(name="xt", bufs=1) as xt_pool,
            tc.tile_pool(name="ob", bufs=4) as ob_pool,
            tc.tile_pool(name="ps", bufs=7, space="PSUM") as ps_pool,
            tc.tile_pool(name="wps", bufs=1, space="PSUM") as wps_pool,
        ):
            tw_t = tw_pool.tile([128, 6 * 96], dt.bfloat16)
            # partitions 0:64 = xp, 64:128 = xp shifted one padded row (+35)
            xt = xt_pool.tile([128, 8, PLANE], dt.bfloat16)

            xf = xt[:].rearrange("p d e -> p (d e)")

            def load_plane(eng, j, d):
                # K-half j: partitions j*64+ci get the plane data shifted by
                # j padded rows (+35 elements); per-partition contiguous so
                # the DMA fans out as one descriptor per partition
                lo, hi = d * PLANE, (d + 1) * PLANE
                return eng.dma_start(xf[j * 64:(j + 1) * 64, lo:hi],
                                     xsf[:, lo + 35 * j:hi + 35 * j])

            # DMA engines round-robin over ALL queued descriptors, so any
            # load's completion lands roughly when everything queued so far
            # finishes. Queue only what the first plane-group needs up
            # front; later planes' loads are gated on earlier groups'
            # matmuls below so in-flight DMA stays ~one plane deep.
            # A stationaries land first (all the first group needs for its
            # first three passes); B's stream in parallel on the other queue
            nc.sync.dma_start(tw_t[:, 0:288], twf[:, 0:288])
            nc.scalar.dma_start(tw_t[:, 288:576], twf[:, 288:576])
            # plane 0 split by rows: group (0, hh=0) only needs rows 0:20,
            # so its matmuls fire before the rest of the plane lands
            nc.scalar.dma_start(xf[0:64, 0:700], xsf[:, 0:700])
            nc.sync.dma_start(xf[64:128, 0:700], xsf[:, 35:735])
            nc.scalar.dma_start(xf[0:64, 700:PLANE], xsf[:, 700:PLANE])
            nc.sync.dma_start(xf[64:128, 700:PLANE],
                              xsf[:, 735:PLANE + 35])

            # PE DVFS warmup: dependency-free matmuls on memset tiles keep
            # the tensor engine busy through its ~3us p-state ramp while the
            # first input DMAs are still in flight, so real matmuls start at
            # full clock. Junk results land in a never-read psum buffer.
            wl = ob_pool.tile([128, 128], dt.bfloat16, tag="wl")
            wr = ob_pool.tile([128, 512], dt.bfloat16, tag="wr")
            nc.gpsimd.memset(wl[:], 0)
            nc.gpsimd.memset(wr[:], 0)
            wps = wps_pool.tile([128, 512], dt.float32)
            prev_last_mm = None
            warm_gate = None
            for wi in range(10):
                wmm = nc.tensor.matmul(wps[:], wl[:], wr[:],
                                       start=True, stop=True)
                if prev_last_mm is not None:
                    tile.add_dep_helper(wmm.ins, prev_last_mm.ins,
                                        sync=False, reason="warmup order")
                prev_last_mm = wmm
                if wi == 3:
                    warm_gate = wmm
            # planes 1-3 stream while the warmup is still ramping so the
            # full-speed consumption of groups d0-d2 never starves
            for dn in (1, 2, 3):
                ld0 = load_plane(nc.scalar, 0, dn)
                ld1 = load_plane(nc.sync, 1, dn)
                for ld in (ld0, ld1):
                    tile.add_dep_helper(ld.ins, warm_gate.ins, sync=True,
                                        reason="throttle input stream")

            xv = xt[:].rearrange("p d (r w) -> p d r w", r=35)

            for d in range(8):
                ob = ob_pool.tile([96, 2, 512], dt.bfloat16)
                for hh in range(2):
                    r0 = 16 * hh
                    ps = ps_pool.tile([96, 16, 32], dt.float32)
                    for i in range(6):
                        ab, kw = i // 3, i % 3
                        s = kw if ab == 0 else 3 + kw
                        rr = r0 + (0 if ab == 0 else 2)
                        rhs = xv[:, d, rr:rr + 16, kw:kw + 32]
                        mm = nc.tensor.matmul(ps[:],
                                              tw_t[:, s * 96:(s + 1) * 96],
                                              rhs, start=(i == 0),
                                              stop=(i == 5))
                        # keep the PE's static order group-contiguous so the
                        # flush fires right after each group's 6th matmul
                        if i == 0 and prev_last_mm is not None:
                            tile.add_dep_helper(
                                mm.ins, prev_last_mm.ins, sync=False,
                                reason="group-contiguous PE order")
                        if i == 0 and hh == 0 and 0 < d + 3 < 8 and d >= 1:
                            dn = d + 3
                            ld0 = load_plane(nc.scalar, 0, dn)
                            ld1 = load_plane(nc.sync, 1, dn)
                            for ld in (ld0, ld1):
                                tile.add_dep_helper(
                                    ld.ins, mm.ins, sync=True,
                                    reason="throttle input stream")
                    prev_last_mm = mm
                    nc.vector.tensor_scalar_add(
                        ob[:, hh, :], ps[:].rearrange("p r w -> p (r w)"),
                        0.0)
                    if d == 7:
                        # split the final plane's store per-half so only the
                        # last 1KB/partition remains after the last matmul
                        nc.sync.dma_start(out[:, d, hh], ob[:, hh, :])
                if d < 7:
                    (nc.scalar if d % 2 == 0 else nc.sync).dma_start(
                        out[:, d], ob[:])

    nc.compile()
    return nc


def _prep_twf(weight):
    import ml_dtypes

    # wc[co, ci, kd, kh, kw] = weight[ci, co, 2-kd, 2-kh, 2-kw]
    wc = np.flip(np.transpose(weight, (1, 0, 2, 3, 4)), axis=(2, 3, 4))
    twf = np.zeros((128, 6, 32, 3), np.float32)
    for kw in range(3):
        for j in range(2):  # A(kw): rows j*64+ci <- kh=j
            twf[j * 64:(j + 1) * 64, kw, :, :] = \
                wc[:, :, :, j, kw].transpose(1, 0, 2)
        # B(kw): rows 0:64 <- kh=2 (K=64 matmul reads only these rows)
        twf[0:64, 3 + kw, :, :] = wc[:, :, :, 2, kw].transpose(1, 0, 2)
    return np.ascontiguousarray(
        twf.reshape(128, 6 * 96)).astype(ml_dtypes.bfloat16)


def _make_slab(x, n, cblk):
    import ml_dtypes

    xs = np.zeros((64, 8 * PLANE + 35), np.float32)
    v = xs[:, :8 * PLANE].reshape(64, 8, 35, 35)
    v[:, :, 1:33, 1:33] = x[n, :, 8 * cblk:8 * cblk + 8]
    return xs.astype(ml_dtypes.bfloat16)


def kernel(x, weight, bias):
    from concourse.bass_utils import run_bass_kernel_spmd

    if "nc" not in _cache:
        _cache["nc"] = _build_nc()
    nc = _cache["nc"]

    x = np.asarray(x, np.float32)
    weight = np.asarray(weight, np.float32)
    bias = np.asarray(bias, np.float32)

    twf = _prep_twf(weight)
    in_maps = []
    for core in range(N_CORES):
        n, cblk = divmod(core, D_BLOCKS)
        in_maps.append({"xsf": _make_slab(x, n, cblk), "twf": twf})

    res = run_bass_kernel_spmd(nc, in_maps, core_ids=list(range(N_CORES)))

    # per-batch global partials P[n, co, kd, d, h, w]
    P = np.empty((2, 32, 3, 32, 32, 32), np.float32)
    for core in range(N_CORES):
        n, cblk = divmod(core, D_BLOCKS)
        arr = np.asarray(res.results[core]["out"], dtype=np.float32)
        # [96, 8, 2, 512] -> (co, kd, d, hh, r, w) -> (co, kd, d, 32, 32)
        arr = arr.reshape(32, 3, 8, 2, 16, 32).reshape(32, 3, 8, 32, 32)
        P[n, :, :, 8 * cblk:8 * cblk + 8] = arr

    # y[m] = P_0[m-1] + P_1[m] + P_2[m+1]
    y = P[:, :, 1].copy()
    y[:, :, 1:] += P[:, :, 0, :31]
    y[:, :, :31] += P[:, :, 2, 1:]
    y += bias[None, :, None, None, None]

    # host assembly: everything except the odd sub-lattice is exactly bias
    full = np.empty((2, 32, 66, 66, 66), np.float32)
    full[...] = bias[None, :, None, None, None]
    full[:, :, 1:64:2, 1:64:2, 1:64:2] = y
    return full


# revision 47
# speedup vs baseline: 1.2559x; 1.0012x over previous
"""ConvTranspose3d(64->32, k=3, stride=2, pad=1, out_pad=1, dilation=2) on 8 NeuronCores.

Math: with stride=2, dilation=2, padding=1, k=3, every populated output
position o = 2i + 2k - 1 is odd in all three spatial dims, so the transposed
conv collapses to a dense 3^3 conv y[m] = sum_k wc[k] * x[m+k-1] on the 32^3
grid (wc = flip(transpose(w))) scattered into the odd sub-lattice of the 66^3
output; every other output voxel is exactly bias.

Device kernel (per core = one (batch, 8-plane depth slab)): for each of its 8
input planes d it computes three 2D-conv partials
    P_kd[co, oh, ow] = sum_{ci,kh,kw} wc[co,ci,kd,kh,kw] * xp[d, oh+kh, ow+kw]
as an implicit GEMM with M = (co*3+kd) = 96 PSUM partitions and K = (64 ci x
2 row-shifted input copies) = 128 SBUF partitions; the row-shifted copy is the
same contiguous DRAM bytes DMA'd at element offset +35 (one padded row), so
taps (kh=0,kh=1) pair up in K and each (plane, h-half) needs only 6 matmuls
of N=512 (3 full A-matmuls kh={0,1} + 3 half B-matmuls kh=2) instead of the
13.5 the depth-Toeplitz formulation needed. No depth halo is loaded; the host
combines P_kd across planes/cores (y[m] = P_0[m-1] + P_1[m] + P_2[m+1]), adds
bias, and scatters into the odd sub-lattice of the 66^3 canvas.
"""

import sys

sys.path.insert(0, "/opt/trn_rl_repo")

import numpy as np

N_CORES = 8
D_BLOCKS = 4   # depth blocks per batch
G_PER_CORE = 8  # input planes per core
PLANE = 35 * 35  # padded plane stride in elements

_cache = {}


def _build_nc():
    import concourse.bass as bass
    import concourse.tile as tile
    from concourse import bacc, mybir

    dt = mybir.dt
    nc = bacc.Bacc("TRN2", target_bir_lowering=False, debug=False,
                   num_devices=N_CORES)

    # xsf[ci, d*1225 + r*35 + w] = padded plane pixel; trailing 35 zeros so
    # the +35 row-shifted alias stays in bounds.
    xsf = nc.dram_tensor("xsf", [64, 8 * PLANE + 35], dt.bfloat16,
                         kind="ExternalInput")
    # 6 stationaries, cols (co*3+kd): s=0..2 A(kw) rows (j*64+ci) = tap
    # (kh=j, kw); s=3..5 B(kw) rows 0:64 = tap (kh=2, kw), K=64 only
    twf = nc.dram_tensor("twf", [128, 6 * 96], dt.bfloat16,
                         kind="ExternalInput")
    # P partials: [co*3+kd, d, hh, 512]
    out = nc.dram_tensor("out", [96, 8, 2, 512], dt.bfloat16,
                         kind="ExternalOutput")

    with tile.TileContext(nc) as tc:
        with (
            tc.tile_pool(name="tw", bufs=1) as tw_pool,
            tc.tile_pool(name="xt", bufs=1) as xt_pool,
            tc.tile_pool(name="ob", bufs=4) as ob_pool,
            tc.tile_pool(name="ps", bufs=7, space="PSUM") as ps_pool,
            tc.tile_pool(name="wps", bufs=1, space="PSUM") as wps_pool,
        ):
            tw_t = tw_pool.tile([128, 6 * 96], dt.bfloat16)
            # partitions 0:64 = xp, 64:128 = xp shifted one padded row (+35)
            xt = xt_pool.tile([128, 8, PLANE], dt.bfloat16)

            xf = xt[:].rearrange("p d e -> p (d e)")

            def load_plane(eng, j, d):
                # K-half j: partitions j*64+ci get the plane data shifted by
                # j padded rows (+35 elements); per-partition contiguous so
                # the DMA fans out as one descriptor per partition
                lo, hi = d * PLANE, (d + 1) * PLANE
                return eng.dma_start(xf[j * 64:(j + 1) * 64, lo:hi],
                                     xsf[:, lo + 35 * j:hi + 35 * j])

            # DMA engines round-robin over ALL queued descriptors, so any
            # load's completion lands roughly when everything queued so far
            # finishes. Queue only what the first plane-group needs up
            # front; later planes' loads are gated on earlier groups'
            # matmuls below so in-flight DMA stays ~one plane deep.
            # A stationaries land first (all the first group needs for its
            # first three passes); B's stream in parallel on the other queue.
            # The first 24KB piece alone unblocks the very first matmul.
            nc.sync.dma_start(tw_t[:, 0:96], twf[:, 0:96])
            nc.sync.dma_start(tw_t[:, 96:288], twf[:, 96:288])
            nc.scalar.dma_start(tw_t[:, 288:576], twf[:, 288:576])
            # plane 0 split by rows: group (0, hh=0) only needs rows 0:20,
            # so its matmuls fire before the rest of the plane lands
            nc.scalar.dma_start(xf[0:64, 0:700], xsf[:, 0:700])
            nc.sync.dma_start(xf[64:128, 0:700], xsf[:, 35:735])
            nc.scalar.dma_start(xf[0:64, 700:PLANE], xsf[:, 700:PLANE])
            nc.sync.dma_start(xf[64:128, 700:PLANE],
                              xsf[:, 735:PLANE + 35])

            # PE DVFS warmup: dependency-free matmuls on memset tiles keep
            # the tensor engine busy through its ~3us p-state ramp while the
            # first input DMAs are still in flight, so real matmuls start at
            # full clock. Junk results land in a never-read psum buffer.
            wl = ob_pool.tile([128, 128], dt.bfloat16, tag="wl")
            wr = ob_pool.tile([128, 512], dt.bfloat16, tag="wr")
            nc.gpsimd.memset(wl[:], 0)
            nc.gpsimd.memset(wr[:], 0)
            wps = wps_pool.tile([128, 512], dt.float32)
            prev_last_mm = None
            warm_gate = None
            for wi in range(10):
                wmm = nc.tensor.matmul(wps[:], wl[:], wr[:],
                                       start=True, stop=True)
                if prev_last_mm is not None:
                    tile.add_dep_helper(wmm.ins, prev_last_mm.ins,
                                        sync=False, reason="warmup order")
                prev_last_mm = wmm
                if wi == 3:
                    warm_gate = wmm
            # planes 1-3 stream while the warmup is still ramping so the
            # full-speed consumption of groups d0-d2 never starves
            for dn in (1, 2, 3):
                ld0 = load_plane(nc.scalar, 0, dn)
                ld1 = load_plane(nc.sync, 1, dn)
                for ld in (ld0, ld1):
                    tile.add_dep_helper(ld.ins, warm_gate.ins, sync=True,
                                        reason="throttle input stream")

            xv = xt[:].rearrange("p d (r w) -> p d r w", r=35)

            for d in range(8):
                ob = ob_pool.tile([96, 2, 512], dt.bfloat16)
                for hh in range(2):
                    r0 = 16 * hh
                    ps = ps_pool.tile([96, 16, 32], dt.float32)
                    for i in range(6):
                        ab, kw = i // 3, i % 3
                        s = kw if ab == 0 else 3 + kw
                        rr = r0 + (0 if ab == 0 else 2)
                        rhs = xv[:, d, rr:rr + 16, kw:kw + 32]
                        mm = nc.tensor.matmul(ps[:],
                                              tw_t[:, s * 96:(s + 1) * 96],
                                              rhs, start=(i == 0),
                                              stop=(i == 5))
                        # keep the PE's static order group-contiguous so the
                        # flush fires right after each group's 6th matmul
                        if i == 0 and prev_last_mm is not None:
                            tile.add_dep_helper(
                                mm.ins, prev_last_mm.ins, sync=False,
                                reason="group-contiguous PE order")
                        if i == 0 and hh == 0 and 0 < d + 3 < 8 and d >= 1:
                            dn = d + 3
                            ld0 = load_plane(nc.scalar, 0, dn)
                            ld1 = load_plane(nc.sync, 1, dn)
                            for ld in (ld0, ld1):
                                tile.add_dep_helper(
                                    ld.ins, mm.ins, sync=True,
                                    reason="throttle input stream")
                    prev_last_mm = mm
                    psf = ps[:].rearrange("p r w -> p (r w)")
                    if d == 7 and hh == 1:
                        # final flush is the critical tail: two half-copies
                        # on DVE and Activation in parallel, each half-store
                        # issued on its own queue as soon as its copy lands
                        nc.vector.tensor_scalar_add(
                            ob[:, hh, 0:256], psf[:, 0:256], 0.0)
                        nc.scalar.copy(ob[:, hh, 256:512], psf[:, 256:512])
                        nc.sync.dma_start(out[:, d, hh, 0:256],
                                          ob[:, hh, 0:256])
                        nc.scalar.dma_start(out[:, d, hh, 256:512],
                                            ob[:, hh, 256:512])
                    else:
                        nc.vector.tensor_scalar_add(ob[:, hh, :], psf, 0.0)
                        if d == 7:
                            nc.sync.dma_start(out[:, d, hh], ob[:, hh, :])
                if d < 7:
                    (nc.scalar if d % 2 == 0 else nc.sync).dma_start(
                        out[:, d], ob[:])

    nc.compile()
    return nc


def _prep_twf(weight):
    import ml_dtypes

    # wc[co, ci, kd, kh, kw] = weight[ci, co, 2-kd, 2-kh, 2-kw]
    wc = np.flip(np.transpose(weight, (1, 0, 2, 3, 4)), axis=(2, 3, 4))
    twf = np.zeros((128, 6, 32, 3), np.float32)
    for kw in range(3):
        for j in range(2):  # A(kw): rows j*64+ci <- kh=j
            twf[j * 64:(j + 1) * 64, kw, :, :] = \
                wc[:, :, :, j, kw].transpose(1, 0, 2)
        # B(kw): rows 0:64 <- kh=2 (K=64 matmul reads only these rows)
        twf[0:64, 3 + kw, :, :] = wc[:, :, :, 2, kw].transpose(1, 0, 2)
    return np.ascontiguousarray(
        twf.reshape(128, 6 * 96)).astype(ml_dtypes.bfloat16)


def _make_slab(x, n, cblk):
    import ml_dtypes

    xs = np.zeros((64, 8 * PLANE + 35), np.float32)
    v = xs[:, :8 * PLANE].reshape(64, 8, 35, 35)
    v[:, :, 1:33, 1:33] = x[n, :, 8 * cblk:8 * cblk + 8]
    return xs.astype(ml_dtypes.bfloat16)


def kernel(x, weight, bias):
    from concourse.bass_utils import run_bass_kernel_spmd

    if "nc" not in _cache:
        _cache["nc"] = _build_nc()
    nc = _cache["nc"]

    x = np.asarray(x, np.float32)
    weight = np.asarray(weight, np.float32)
    bias = np.asarray(bias, np.float32)

    twf = _prep_twf(weight)
    in_maps = []
    for core in range(N_CORES):
        n, cblk = divmod(core, D_BLOCKS)
        in_maps.append({"xsf": _make_slab(x, n, cblk), "twf": twf})

    res = run_bass_kernel_spmd(nc, in_maps, core_ids=list(range(N_CORES)))

    # per-batch global partials P[n, co, kd, d, h, w]
    P = np.empty((2, 32, 3, 32, 32, 32), np.float32)
    for core in range(N_CORES):
        n, cblk = divmod(core, D_BLOCKS)
        arr = np.asarray(res.results[core]["out"], dtype=np.float32)
        # [96, 8, 2, 512] -> (co, kd, d, hh, r, w) -> (co, kd, d, 32, 32)
        arr = arr.reshape(32, 3, 8, 2, 16, 32).reshape(32, 3, 8, 32, 32)
        P[n, :, :, 8 * cblk:8 * cblk + 8] = arr

    # y[m] = P_0[m-1] + P_1[m] + P_2[m+1]
    y = P[:, :, 1].copy()
    y[:, :, 1:] += P[:, :, 0, :31]
    y[:, :, :31] += P[:, :, 2, 1:]
    y += bias[None, :, None, None, None]

    # host assembly: everything except the odd sub-lattice is exactly bias
    full = np.empty((2, 32, 66, 66, 66), np.float32)
    full[...] = bias[None, :, None, None, None]
    full[:, :, 1:64:2, 1:64:2, 1:64:2] = y
    return full
